# revision 33
# baseline (speedup 1.0000x reference)
"""Multi-head dot-product attention (RoPE, causal) on 8 NeuronCores.

Sharding: data-parallel over batch (2) x tensor-parallel over heads (16 -> 4
per core). Each core projects q/k/v for its 4 heads, runs causal attention,
and computes a partial output projection; the host sums the 4 partials per
batch element.

v2 design notes (vs the f32r baseline):
- All matmul operands are bf16 (PSUM accumulation stays f32): same PE
  throughput as f32r but half the DMA bytes and SBUF footprint. Host packs
  every DRAM tensor in the exact SBUF layout so all loads are full-line
  contiguous copies.
- Phase order: KV projection (all 4 t-blocks) -> per t-block [Q projection +
  attention of the previous t-block interleaved] -> output projection. The
  PE instruction stream never waits on a phase boundary: attention t-block
  tb only needs KV blocks <= tb and the Q block produced just before it.
- Attention keeps the transposed-scores layout: ST[s, t] so A@V needs no
  transposes, additive causal masks on the 4 diagonal sub-blocks only
  (width-trimmed), softmax denominator via an all-ones stationary matmul,
  reciprocal via Ln/Exp on the scalar engine (same activation table set as
  the softmax Exp), normalization during PSUM eviction.
- Cross-head interleave: the PE stream runs QK(h+1) between QK(h) and
  den/AV(h) so the scalar engine's exp latency is hidden; two eG buffers
  (even/odd head) break the WAR chain between consecutive heads.
- PSUM: q-projection accumulators share the attention score pool slots
  (3 x [128,2,512] = 6 banks) + den (1) + AV (1) = 8 banks exactly.
"""

import numpy as np

B, S, E, N, D = 2, 2048, 2048, 16, 128
HL = 4           # local heads per core (8 cores = 2 batch x 4 head groups)
ND = HL * D      # 512
NT = S // 128    # 16 row tiles
NB = S // 512    # 4 row blocks
NE = E // 128    # 16 contraction tiles
MASK_VALUE = float(-0.7 * np.finfo(np.float32).max)
MW = [128, 256, 384, 512]            # mask widths per diagonal variant
MOFF = [0, 128, 384, 768]            # col offsets of variants in msk table

_NC_CACHE = {}


def _build_module():
    import concourse.bass as bass
    import concourse.mybir as mybir
    import concourse.tile as tile
    from concourse import bacc

    f32 = mybir.dt.float32
    bf16 = mybir.dt.bfloat16
    Exp = mybir.ActivationFunctionType.Exp

    nc = bacc.Bacc("TRN2", target_bir_lowering=False, debug=False, num_devices=8)

    # Host-packed layouts (flat contiguous DMAs):
    xq_d = nc.dram_tensor("xq_p", [NB, 128, NE, 512], bf16, kind="ExternalInput").ap()
    xkv_d = nc.dram_tensor("xkv_p", [NB, 128, NE, 512], bf16, kind="ExternalInput").ap()
    wq_d = nc.dram_tensor("wq_p", [128, NE, ND], bf16, kind="ExternalInput").ap()
    wk_d = nc.dram_tensor("wk_p", [128, NE, ND], bf16, kind="ExternalInput").ap()
    wv_d = nc.dram_tensor("wv_p", [128, NE, ND], bf16, kind="ExternalInput").ap()
    wo_d = nc.dram_tensor("wo_p", [128, HL, E], bf16, kind="ExternalInput").ap()
    csd_d = nc.dram_tensor("csd", [128, S], f32, kind="ExternalInput").ap()
    sns_d = nc.dram_tensor("sns", [128, S], f32, kind="ExternalInput").ap()
    ones_d = nc.dram_tensor("ones", [128, 128], bf16, kind="ExternalInput").ap()
    msk_d = nc.dram_tensor("msk", [128, 1280], f32, kind="ExternalInput").ap()
    out_d = nc.dram_tensor("out", [NT, 128, E], bf16, kind="ExternalOutput").ap()

    with tile.TileContext(nc) as tc:
        with tc.tile_pool(name="const", bufs=1) as cpool, \
             tc.tile_pool(name="wqo", bufs=1) as wqo_pool, \
             tc.tile_pool(name="xq", bufs=2) as xq_pool, \
             tc.tile_pool(name="persist", bufs=1) as pers_pool:
            csd = cpool.tile([128, S], f32, tag="csd")
            sns = cpool.tile([128, S], f32, tag="sns")
            msk = cpool.tile([128, 1280], f32, tag="msk")
            ones = cpool.tile([128, 128], bf16, tag="ones")
            wq = wqo_pool.tile([128, NE, ND], bf16, tag="wq")
            wo = wqo_pool.tile([128, HL, E], bf16, tag="wo")
            kT = [pers_pool.tile([128, S], bf16, tag=f"kT{h}", name=f"kT{h}")
                  for h in range(HL)]
            vG = [pers_pool.tile([128, 4, ND], bf16, tag=f"vG{g}", name=f"vG{g}")
                  for g in range(NB)]
            uT = [pers_pool.tile([128, S], bf16, tag=f"uT{h}", name=f"uT{h}")
                  for h in range(HL)]
            xq_tiles = {}

            def load_xq(tb):
                xqt = xq_pool.tile([128, NE, 512], bf16, tag="xq",
                                   name=f"xq{tb}")
                nc.sync.dma_start(xqt[:].rearrange("p a b -> p (a b)"),
                                  xq_d[tb].rearrange("p a b -> p (a b)"))
                xq_tiles[tb] = xqt

            def rope(dst, src, tb, pool):
                """dst[:, tb-block] = rope(src) with de-interleaved head dim.
                src is a [128, 512] f32 PSUM AP; dst is bf16 SBUF."""
                tbs = bass.ts(tb, 512)
                tmp = pool.tile([128, 512], f32, tag="tmp", name="tmp")
                tmp2 = pool.tile([128, 512], f32, tag="tmp2", name="tmp2")
                nc.vector.tensor_mul(tmp[0:64, :], src[64:128, :], sns[0:64, tbs])
                nc.vector.tensor_mul(tmp[64:128, :], src[0:64, :], sns[64:128, tbs])
                nc.vector.tensor_mul(tmp2[:], src[:], csd[:, tbs])
                nc.vector.tensor_add(dst[:, tbs], tmp2[:], tmp[:])

            # ---------------- Phase 1: K + V projection ----------------
            with nc.named_scope("proj_kv"), \
                 tc.tile_pool(name="wkv", bufs=1) as wkv_pool, \
                 tc.tile_pool(name="xkv", bufs=2) as xkv_pool, \
                 tc.tile_pool(name="kvps", bufs=2, space="PSUM") as kvps_pool, \
                 tc.tile_pool(name="rope_kv", bufs=2) as rkv_pool:
                wk = wkv_pool.tile([128, NE, ND], bf16, tag="wk")
                wv = wkv_pool.tile([128, NE, ND], bf16, tag="wv")
                # Preloads. wk/wv interleaved chunks on the gpsimd queue (the
                # et loop consumes both in lockstep); tables on the scalar
                # queue in parallel; wq/wo behind wk/wv (needed later).
                # Flatten [p, a, b] -> [p, (a b)] on both sides: the DRAM and
                # SBUF runs are contiguous per partition, and 2D APs let the
                # descriptor generator emit 8-16KB descriptors instead of 1KB
                # (1KB descriptors cap HBM DMA at ~40% of peak).
                def fl(ap):
                    return ap.rearrange("p a b -> p (a b)")

                # Start-of-kernel DMA is the critical path: only wk/wv (gpsimd
                # queue) and xkv[0] (sync queue) compete for the engines; the
                # rope tables queue behind the weights, and everything not
                # needed before attention (wq, wo, msk, ones) is dispatched
                # one t-block later.
                for ch in range(4):
                    nc.gpsimd.dma_start(fl(wk[:, 4 * ch:4 * (ch + 1), :]),
                                        fl(wk_d[:, 4 * ch:4 * (ch + 1), :]))
                    nc.gpsimd.dma_start(fl(wv[:, 4 * ch:4 * (ch + 1), :]),
                                        fl(wv_d[:, 4 * ch:4 * (ch + 1), :]))
                nc.gpsimd.dma_start(csd[:], csd_d[:])
                nc.gpsimd.dma_start(sns[:], sns_d[:])

                for tb in range(NB):
                    xk = xkv_pool.tile([128, NE, 512], bf16, tag="xk",
                                       name=f"xk{tb}")
                    if tb == 0:
                        for ch in range(4):
                            nc.sync.dma_start(
                                fl(xk[:, 4 * ch:4 * (ch + 1), :]),
                                fl(xkv_d[tb][:, 4 * ch:4 * (ch + 1), :]))
                    else:
                        nc.sync.dma_start(fl(xk[:]), fl(xkv_d[tb]))
                    if tb == 1:
                        nc.gpsimd.dma_start(fl(wq[:]), fl(wq_d[:]))
                        nc.gpsimd.dma_start(fl(wo[:]), fl(wo_d[:]))
                        nc.scalar.dma_start(msk[:], msk_d[:])
                        nc.scalar.dma_start(ones[:], ones_d[:])
                    if tb == NB - 1:
                        # Dispatch the first two Q blocks behind the last xkv
                        # block on the sync queue so Q projection never waits.
                        load_xq(0)
                        load_xq(1)
                    for pp in range(2):   # 2 half-passes: 2 heads + 2 s-subtiles
                        kps = kvps_pool.tile([128, 2, 512], f32, tag="kps",
                                             name=f"kps{tb}{pp}")
                        vps = kvps_pool.tile([128, 2, 512], f32, tag="vps",
                                             name=f"vps{tb}{pp}")
                        for et in range(NE):
                            for i in range(2):
                                h = 2 * pp + i
                                nc.tensor.matmul(
                                    kps[:, i], wk[:, et, bass.ts(h, 128)],
                                    xk[:, et, :], start=(et == 0),
                                    stop=(et == NE - 1))
                            for i in range(2):
                                sv = 2 * pp + i
                                nc.tensor.matmul(
                                    vps[:, i], xk[:, et, bass.ts(sv, 128)],
                                    wv[:, et, :], start=(et == 0),
                                    stop=(et == NE - 1))
                        for i in range(2):
                            rope(kT[2 * pp + i], kps[:, i], tb, rkv_pool)
                            nc.scalar.copy(vG[tb][:, 2 * pp + i, :], vps[:, i])

            # ---------- Phase 2+3: Q projection + attention, interleaved ----------
            with nc.named_scope("q_attn"), \
                 tc.tile_pool(name="qat", bufs=1) as qat_pool, \
                 tc.tile_pool(name="sps", bufs=3, space="PSUM") as sps_pool, \
                 tc.tile_pool(name="dps", bufs=1, space="PSUM") as dps_pool, \
                 tc.tile_pool(name="ups", bufs=1, space="PSUM") as ups_pool, \
                 tc.tile_pool(name="rope_q", bufs=1) as rq_pool, \
                 tc.tile_pool(name="rcp", bufs=2) as rcp_pool:
                qT = [qat_pool.tile([128, S], bf16, tag=f"qT{h}", name=f"qT{h}")
                      for h in range(HL)]
                # three eG sets, rotating h%3: lets QK of 3 heads run ahead
                # of the first den/AV pass without WAR serialization
                eG = [[qat_pool.tile([128, 2048], bf16, tag=f"eG{p}{g}",
                                     name=f"eG{p}{g}") for g in range(4)]
                      for p in range(3)]

                def e_ap(eset, si):
                    return eset[si // 4][:, bass.ds(512 * (si % 4), 512)]

                def q_mm(tb, hp):
                    """Project heads (2hp, 2hp+1) for t-block tb. Returns psum."""
                    qps = sps_pool.tile([128, 2, 512], f32, tag="sp",
                                        name=f"qps{tb}{hp}")
                    xqt = xq_tiles[tb]
                    for et in range(NE):
                        for i in range(2):
                            h = 2 * hp + i
                            nc.tensor.matmul(
                                qps[:, i], wq[:, et, bass.ts(h, 128)],
                                xqt[:, et, :], start=(et == 0),
                                stop=(et == NE - 1))
                    return qps

                def q_rope(tb, hp, qps):
                    for i in range(2):
                        rope(qT[2 * hp + i], qps[:, i], tb, rq_pool)

                def attn_qk(tb, h):
                    """Scores + exp for head h of t-block tb."""
                    nsi = 4 * (tb + 1)
                    eset = eG[h % 3]
                    for j in range(nsi // 2):
                        sp = sps_pool.tile([128, 2, 512], f32, tag="sp",
                                           name=f"sp{tb}{h}{j}")
                        for p2 in range(2):
                            si = 2 * j + p2
                            nc.tensor.matmul(
                                sp[:, p2], kT[h][:, bass.ts(si, 128)],
                                qT[h][:, bass.ts(tb, 512)], start=True,
                                stop=True)
                            v = si - 4 * tb
                            if v >= 0:
                                w = MW[v]
                                nc.vector.tensor_add(
                                    sp[:, p2, 0:w], sp[:, p2, 0:w],
                                    msk[:, bass.ds(MOFF[v], w)])
                        nc.scalar.activation(
                            eset[j // 2][:, bass.ts(j % 2, 1024)],
                            sp[:].rearrange("p a b -> p (a b)"), Exp)

                def attn_dv(tb, h):
                    """Denominator + A@V + normalization for head h."""
                    nsi = 4 * (tb + 1)
                    eset = eG[h % 3]
                    den = dps_pool.tile([128, 512], f32, tag="den",
                                        name=f"den{tb}{h}")
                    for si in range(nsi):
                        nc.tensor.matmul(den[:], ones[:], e_ap(eset, si),
                                         start=(si == 0), stop=(si == nsi - 1))
                    rec = rcp_pool.tile([128, 512], f32, tag="rec", name="rec")
                    nc.vector.reciprocal(rec[:], den[:])
                    up = ups_pool.tile([128, 512], f32, tag="up",
                                       name=f"up{tb}{h}")
                    for si in range(nsi):
                        g, sv = si // 4, si % 4
                        nc.tensor.matmul(up[:], vG[g][:, sv, bass.ts(h, 128)],
                                         e_ap(eset, si), start=(si == 0),
                                         stop=(si == nsi - 1))
                    nc.vector.tensor_mul(uT[h][:, bass.ts(tb, 512)], up[:],
                                         rec[:])

                def attn_block(tb, qnext=None, ropes_mid=None):
                    """Full attention t-block. The PE stream runs 3 QK chains
                    ahead of the first den/AV pass so the scalar engine's exp
                    stream (573ns/slice vs the PE's 216) never gates the PE;
                    the next t-block's first Q chain covers the den(3) tail.
                    ropes_mid (the next block's second rope pair) is placed
                    after qk1 so the DVE processes this block's diagonal mask
                    adds first — exp of the diagonal pairs gates den()."""
                    attn_qk(tb, 0)
                    attn_qk(tb, 1)
                    if ropes_mid is not None:
                        ropes_mid()
                    attn_qk(tb, 2)
                    attn_dv(tb, 0)
                    attn_qk(tb, 3)
                    attn_dv(tb, 1)
                    attn_dv(tb, 2)
                    if qnext is not None:
                        qnext()
                    attn_dv(tb, 3)

                qps = q_mm(0, 0)
                q_rope(0, 0, qps)
                qps = q_mm(0, 1)
                q_rope(0, 1, qps)
                ropes_mid = None
                for tb in range(1, NB):
                    if tb + 1 < NB:
                        load_xq(tb + 1)
                    holder = {}

                    def qnext(tb=tb, holder=holder):
                        holder["qps"] = q_mm(tb, 0)

                    attn_block(tb - 1, qnext=qnext, ropes_mid=ropes_mid)
                    q_rope(tb, 0, holder["qps"])
                    qps1 = q_mm(tb, 1)

                    def ropes_mid(tb=tb, qps1=qps1):
                        q_rope(tb, 1, qps1)

                attn_block(NB - 1, ropes_mid=ropes_mid)

            # ---------------- Phase 4: output projection ----------------
            with nc.named_scope("out_proj"), \
                 tc.tile_pool(name="ops", bufs=2, space="PSUM") as ops_pool, \
                 tc.tile_pool(name="ob", bufs=3) as ob_pool:
                for tt in range(NT):
                    op = ops_pool.tile([128, E], f32, tag="op", name=f"op{tt}")
                    for ec in range(4):
                        for h in range(HL):
                            nc.tensor.matmul(
                                op[:, bass.ts(ec, 512)],
                                uT[h][:, bass.ts(tt, 128)],
                                wo[:, h, bass.ts(ec, 512)],
                                start=(h == 0), stop=(h == HL - 1))
                    ob = ob_pool.tile([128, E], bf16, tag="ob", name=f"ob{tt}")
                    nc.scalar.copy(ob[:], op[:])
                    # Alternate store queues: the 8MB output stream exceeds
                    # one queue's bandwidth over this phase's 50us window.
                    q = nc.sync if tt % 2 == 0 else nc.gpsimd
                    q.dma_start(out_d[tt], ob[:])

    nc.compile()
    return nc


def _get_module():
    if "nc" not in _NC_CACHE:
        _NC_CACHE["nc"] = _build_module()
    return _NC_CACHE["nc"]


def _host_prep(inputs_q, inputs_kv, positions, Wq, Wk, Wv, Wo):
    """Build the 8 per-core input maps (device-packed layouts, bf16)."""
    import ml_dtypes
    bf16 = ml_dtypes.bfloat16
    perm = np.concatenate([np.arange(0, D, 2), np.arange(1, D, 2)])  # de-interleave
    scale = np.float32(1.0 / np.sqrt(D))
    half = D // 2
    timescale = 10000.0 ** (2.0 * np.arange(half, dtype=np.float64) / D)
    ones = np.ones((128, 128), dtype=bf16)
    # mask variant v (diag sub-block at cols [128v, 128v+128)), width-trimmed:
    # masked (additive MASK_VALUE) where col < 128*v + row
    s_i = np.arange(128)[:, None]
    msk = np.concatenate(
        [np.where(np.arange(MW[v])[None, :] < 128 * v + s_i, MASK_VALUE, 0.0)
         for v in range(4)], axis=1).astype(np.float32)

    def pack_x(xT):
        # [E, S] f32 -> [NB, 128, NE, 512]: x_p[tb, p, et, t] = xT[128 et + p, 512 tb + t]
        return np.ascontiguousarray(
            xT.reshape(NE, 128, NB, 512).transpose(2, 1, 0, 3).astype(bf16))

    def pack_w(w):
        # [E, ND] -> [128, NE, ND]: w_p[p, et, n] = w[128 et + p, n]
        return np.ascontiguousarray(
            w.reshape(NE, 128, ND).transpose(1, 0, 2).astype(bf16))

    in_maps = []
    for c in range(8):
        b = c // 4
        h0 = (c % 4) * HL
        angle = positions[b].astype(np.float64)[None, :] / timescale[:, None]  # [64,S]
        cs = np.cos(angle).astype(np.float32)
        sn = np.sin(angle).astype(np.float32)
        csd = np.concatenate([cs, cs], axis=0)               # [128, S]
        sns = np.concatenate([-sn, sn], axis=0)              # [128, S]
        wq = (Wq[:, h0:h0 + HL, :][:, :, perm] * scale).reshape(E, ND)
        wk = Wk[:, h0:h0 + HL, :][:, :, perm].reshape(E, ND)
        wv = Wv[:, h0:h0 + HL, :].reshape(E, ND)
        wo = Wo[h0:h0 + HL]                                   # [HL, D, E]
        in_maps.append({
            "xq_p": pack_x(np.asarray(inputs_q[b]).T),
            "xkv_p": pack_x(np.asarray(inputs_kv[b]).T),
            "wq_p": pack_w(np.asarray(wq, dtype=np.float32)),
            "wk_p": pack_w(np.asarray(wk, dtype=np.float32)),
            "wv_p": pack_w(np.asarray(wv, dtype=np.float32)),
            "wo_p": np.ascontiguousarray(
                np.asarray(wo, dtype=np.float32).transpose(1, 0, 2).astype(bf16)),
            "csd": csd, "sns": sns, "ones": ones, "msk": msk,
        })
    return in_maps


def kernel(inputs_q, inputs_kv, positions, Wq, Wk, Wv, Wo, _trace=False,
           _trace_kwargs=None):
    from concourse import bass_utils

    nc = _get_module()
    in_maps = _host_prep(inputs_q, inputs_kv, positions, Wq, Wk, Wv, Wo)
    res = bass_utils.run_bass_kernel_spmd(
        nc, in_maps, core_ids=list(range(8)), trace=_trace,
        **(_trace_kwargs or {}))
    if _trace:
        _NC_CACHE["last_results"] = res
    parts = [np.asarray(res.results[c]["out"], dtype=np.float32).reshape(S, E)
             for c in range(8)]
    out0 = parts[0] + parts[1] + parts[2] + parts[3]
    out1 = parts[4] + parts[5] + parts[6] + parts[7]
    return np.stack([out0, out1]).astype(np.float32)


# revision 36
# speedup vs baseline: 1.0108x; 1.0108x over previous
"""Multi-head dot-product attention (RoPE, causal) on 8 NeuronCores.

Sharding: data-parallel over batch (2) x tensor-parallel over heads (16 -> 4
per core). Each core projects q/k/v for its 4 heads, runs causal attention,
and computes a partial output projection; the host sums the 4 partials per
batch element.

v2 design notes (vs the f32r baseline):
- All matmul operands are bf16 (PSUM accumulation stays f32): same PE
  throughput as f32r but half the DMA bytes and SBUF footprint. Host packs
  every DRAM tensor in the exact SBUF layout so all loads are full-line
  contiguous copies.
- Phase order: KV projection (all 4 t-blocks) -> per t-block [Q projection +
  attention of the previous t-block interleaved] -> output projection. The
  PE instruction stream never waits on a phase boundary: attention t-block
  tb only needs KV blocks <= tb and the Q block produced just before it.
- Attention keeps the transposed-scores layout: ST[s, t] so A@V needs no
  transposes, additive causal masks on the 4 diagonal sub-blocks only
  (width-trimmed), softmax denominator via an all-ones stationary matmul,
  reciprocal via Ln/Exp on the scalar engine (same activation table set as
  the softmax Exp), normalization during PSUM eviction.
- Cross-head interleave: the PE stream runs QK(h+1) between QK(h) and
  den/AV(h) so the scalar engine's exp latency is hidden; two eG buffers
  (even/odd head) break the WAR chain between consecutive heads.
- PSUM: q-projection accumulators share the attention score pool slots
  (3 x [128,2,512] = 6 banks) + den (1) + AV (1) = 8 banks exactly.
"""

import numpy as np

B, S, E, N, D = 2, 2048, 2048, 16, 128
HL = 4           # local heads per core (8 cores = 2 batch x 4 head groups)
ND = HL * D      # 512
NT = S // 128    # 16 row tiles
NB = S // 512    # 4 row blocks
NE = E // 128    # 16 contraction tiles
MASK_VALUE = float(-0.7 * np.finfo(np.float32).max)
MW = [128, 256, 384, 512]            # mask widths per diagonal variant
MOFF = [0, 128, 384, 768]            # col offsets of variants in msk table

_NC_CACHE = {}


def _build_module():
    import concourse.bass as bass
    import concourse.mybir as mybir
    import concourse.tile as tile
    from concourse import bacc

    f32 = mybir.dt.float32
    bf16 = mybir.dt.bfloat16
    Exp = mybir.ActivationFunctionType.Exp

    nc = bacc.Bacc("TRN2", target_bir_lowering=False, debug=False, num_devices=8)

    # Host-packed layouts (flat contiguous DMAs):
    xq_d = nc.dram_tensor("xq_p", [NB, 128, NE, 512], bf16, kind="ExternalInput").ap()
    xkv_d = nc.dram_tensor("xkv_p", [NB, 128, NE, 512], bf16, kind="ExternalInput").ap()
    wq_d = nc.dram_tensor("wq_p", [128, NE, ND], bf16, kind="ExternalInput").ap()
    wk_d = nc.dram_tensor("wk_p", [128, NE, ND], bf16, kind="ExternalInput").ap()
    wv_d = nc.dram_tensor("wv_p", [128, NE, ND], bf16, kind="ExternalInput").ap()
    wo_d = nc.dram_tensor("wo_p", [128, HL, E], bf16, kind="ExternalInput").ap()
    csd_d = nc.dram_tensor("csd", [128, S], f32, kind="ExternalInput").ap()
    sns_d = nc.dram_tensor("sns", [128, S], f32, kind="ExternalInput").ap()
    ones_d = nc.dram_tensor("ones", [128, 128], bf16, kind="ExternalInput").ap()
    msk_d = nc.dram_tensor("msk", [128, 1280], f32, kind="ExternalInput").ap()
    out_d = nc.dram_tensor("out", [NT, 128, E], bf16, kind="ExternalOutput").ap()

    with tile.TileContext(nc) as tc:
        with tc.tile_pool(name="const", bufs=1) as cpool, \
             tc.tile_pool(name="wqo", bufs=1) as wqo_pool, \
             tc.tile_pool(name="xq", bufs=2) as xq_pool, \
             tc.tile_pool(name="persist", bufs=1) as pers_pool:
            csd = cpool.tile([128, S], f32, tag="csd")
            sns = cpool.tile([128, S], f32, tag="sns")
            msk = cpool.tile([128, 1280], f32, tag="msk")
            ones = cpool.tile([128, 128], bf16, tag="ones")
            wq = wqo_pool.tile([128, NE, ND], bf16, tag="wq")
            wo = wqo_pool.tile([128, HL, E], bf16, tag="wo")
            kT = [pers_pool.tile([128, S], bf16, tag=f"kT{h}", name=f"kT{h}")
                  for h in range(HL)]
            vG = [pers_pool.tile([128, 4, ND], bf16, tag=f"vG{g}", name=f"vG{g}")
                  for g in range(NB)]
            uT = [pers_pool.tile([128, S], bf16, tag=f"uT{h}", name=f"uT{h}")
                  for h in range(HL)]
            xq_tiles = {}

            def load_xq(tb):
                xqt = xq_pool.tile([128, NE, 512], bf16, tag="xq",
                                   name=f"xq{tb}")
                nc.sync.dma_start(xqt[:].rearrange("p a b -> p (a b)"),
                                  xq_d[tb].rearrange("p a b -> p (a b)"))
                xq_tiles[tb] = xqt

            def rope(dst, src, tb, pool):
                """dst[:, tb-block] = rope(src) with de-interleaved head dim.
                src is a [128, 512] f32 PSUM AP; dst is bf16 SBUF."""
                tbs = bass.ts(tb, 512)
                tmp = pool.tile([128, 512], f32, tag="tmp", name="tmp")
                tmp2 = pool.tile([128, 512], f32, tag="tmp2", name="tmp2")
                nc.vector.tensor_mul(tmp[0:64, :], src[64:128, :], sns[0:64, tbs])
                nc.vector.tensor_mul(tmp[64:128, :], src[0:64, :], sns[64:128, tbs])
                nc.vector.tensor_mul(tmp2[:], src[:], csd[:, tbs])
                nc.vector.tensor_add(dst[:, tbs], tmp2[:], tmp[:])

            # ---------------- Phase 1: K + V projection ----------------
            with nc.named_scope("proj_kv"), \
                 tc.tile_pool(name="wkv", bufs=1) as wkv_pool, \
                 tc.tile_pool(name="xkv", bufs=2) as xkv_pool, \
                 tc.tile_pool(name="kvps", bufs=2, space="PSUM") as kvps_pool, \
                 tc.tile_pool(name="rope_kv", bufs=2) as rkv_pool:
                wk = wkv_pool.tile([128, NE, ND], bf16, tag="wk")
                wv = wkv_pool.tile([128, NE, ND], bf16, tag="wv")
                # Preloads. wk/wv interleaved chunks on the gpsimd queue (the
                # et loop consumes both in lockstep); tables on the scalar
                # queue in parallel; wq/wo behind wk/wv (needed later).
                # Flatten [p, a, b] -> [p, (a b)] on both sides: the DRAM and
                # SBUF runs are contiguous per partition, and 2D APs let the
                # descriptor generator emit 8-16KB descriptors instead of 1KB
                # (1KB descriptors cap HBM DMA at ~40% of peak).
                def fl(ap):
                    return ap.rearrange("p a b -> p (a b)")

                # Start-of-kernel DMA is the critical path: only wk/wv (gpsimd
                # queue) and xkv[0] (sync queue) compete for the engines; the
                # rope tables queue behind the weights, and everything not
                # needed before attention (wq, wo, msk, ones) is dispatched
                # one t-block later.
                nc.sync.dma_start(ones[:], ones_d[:])
                for ch in range(4):
                    nc.gpsimd.dma_start(fl(wk[:, 4 * ch:4 * (ch + 1), :]),
                                        fl(wk_d[:, 4 * ch:4 * (ch + 1), :]))
                    nc.gpsimd.dma_start(fl(wv[:, 4 * ch:4 * (ch + 1), :]),
                                        fl(wv_d[:, 4 * ch:4 * (ch + 1), :]))

                for tb in range(NB):
                    xk = xkv_pool.tile([128, NE, 512], bf16, tag="xk",
                                       name=f"xk{tb}")
                    if tb == 0:
                        for ch in range(4):
                            nc.sync.dma_start(
                                fl(xk[:, 4 * ch:4 * (ch + 1), :]),
                                fl(xkv_d[tb][:, 4 * ch:4 * (ch + 1), :]))
                        # rope tables ride the sync queue behind the first x
                        # block (the gpsimd queue is saturated with weights)
                        nc.sync.dma_start(csd[:], csd_d[:])
                        nc.sync.dma_start(sns[:], sns_d[:])
                    else:
                        nc.sync.dma_start(fl(xk[:]), fl(xkv_d[tb]))
                    if tb == 1:
                        nc.gpsimd.dma_start(fl(wq[:]), fl(wq_d[:]))
                        nc.gpsimd.dma_start(fl(wo[:]), fl(wo_d[:]))
                        nc.scalar.dma_start(msk[:], msk_d[:])
                    if tb == NB - 1:
                        # Dispatch the first two Q blocks behind the last xkv
                        # block on the sync queue so Q projection never waits.
                        load_xq(0)
                        load_xq(1)
                    for pp in range(2):   # 2 half-passes: 2 heads + 2 s-subtiles
                        kps = kvps_pool.tile([128, 2, 512], f32, tag="kps",
                                             name=f"kps{tb}{pp}")
                        vps = kvps_pool.tile([128, 2, 512], f32, tag="vps",
                                             name=f"vps{tb}{pp}")
                        if tb == 0 and pp == 0:
                            # Warm-up: the PE inevitably waits ~10us here for
                            # the first weight/x chunks. Chew dummy matmuls on
                            # the ones tile meanwhile so the HAM clock-gate is
                            # at 2.4GHz when the real chains start (the real
                            # et=0 matmul's start=True resets the bank).
                            for _ in range(40):
                                nc.tensor.matmul(kps[:, 0, 0:128], ones[:],
                                                 ones[:], start=True,
                                                 stop=True)
                        for et in range(NE):
                            for i in range(2):
                                h = 2 * pp + i
                                nc.tensor.matmul(
                                    kps[:, i], wk[:, et, bass.ts(h, 128)],
                                    xk[:, et, :], start=(et == 0),
                                    stop=(et == NE - 1))
                            for i in range(2):
                                sv = 2 * pp + i
                                nc.tensor.matmul(
                                    vps[:, i], xk[:, et, bass.ts(sv, 128)],
                                    wv[:, et, :], start=(et == 0),
                                    stop=(et == NE - 1))
                        for i in range(2):
                            rope(kT[2 * pp + i], kps[:, i], tb, rkv_pool)
                            nc.scalar.copy(vG[tb][:, 2 * pp + i, :], vps[:, i])

            # ---------- Phase 2+3: Q projection + attention, interleaved ----------
            with nc.named_scope("q_attn"), \
                 tc.tile_pool(name="qat", bufs=1) as qat_pool, \
                 tc.tile_pool(name="sps", bufs=3, space="PSUM") as sps_pool, \
                 tc.tile_pool(name="dps", bufs=1, space="PSUM") as dps_pool, \
                 tc.tile_pool(name="ups", bufs=1, space="PSUM") as ups_pool, \
                 tc.tile_pool(name="rope_q", bufs=1) as rq_pool, \
                 tc.tile_pool(name="rcp", bufs=2) as rcp_pool:
                qT = [qat_pool.tile([128, S], bf16, tag=f"qT{h}", name=f"qT{h}")
                      for h in range(HL)]
                # three eG sets, rotating h%3: lets QK of 3 heads run ahead
                # of the first den/AV pass without WAR serialization
                eG = [[qat_pool.tile([128, 2048], bf16, tag=f"eG{p}{g}",
                                     name=f"eG{p}{g}") for g in range(4)]
                      for p in range(3)]

                def e_ap(eset, si):
                    return eset[si // 4][:, bass.ds(512 * (si % 4), 512)]

                def q_mm(tb, hp):
                    """Project heads (2hp, 2hp+1) for t-block tb. Returns psum."""
                    qps = sps_pool.tile([128, 2, 512], f32, tag="sp",
                                        name=f"qps{tb}{hp}")
                    xqt = xq_tiles[tb]
                    for et in range(NE):
                        for i in range(2):
                            h = 2 * hp + i
                            nc.tensor.matmul(
                                qps[:, i], wq[:, et, bass.ts(h, 128)],
                                xqt[:, et, :], start=(et == 0),
                                stop=(et == NE - 1))
                    return qps

                def q_rope(tb, hp, qps):
                    for i in range(2):
                        rope(qT[2 * hp + i], qps[:, i], tb, rq_pool)

                def attn_qk(tb, h):
                    """Scores + exp for head h of t-block tb."""
                    nsi = 4 * (tb + 1)
                    eset = eG[h % 3]
                    for j in range(nsi // 2):
                        sp = sps_pool.tile([128, 2, 512], f32, tag="sp",
                                           name=f"sp{tb}{h}{j}")
                        for p2 in range(2):
                            si = 2 * j + p2
                            nc.tensor.matmul(
                                sp[:, p2], kT[h][:, bass.ts(si, 128)],
                                qT[h][:, bass.ts(tb, 512)], start=True,
                                stop=True)
                            v = si - 4 * tb
                            if v >= 0:
                                w = MW[v]
                                nc.vector.tensor_add(
                                    sp[:, p2, 0:w], sp[:, p2, 0:w],
                                    msk[:, bass.ds(MOFF[v], w)])
                        nc.scalar.activation(
                            eset[j // 2][:, bass.ts(j % 2, 1024)],
                            sp[:].rearrange("p a b -> p (a b)"), Exp)

                def attn_dv(tb, h):
                    """Denominator + A@V + normalization for head h."""
                    nsi = 4 * (tb + 1)
                    eset = eG[h % 3]
                    den = dps_pool.tile([128, 512], f32, tag="den",
                                        name=f"den{tb}{h}")
                    for si in range(nsi):
                        nc.tensor.matmul(den[:], ones[:], e_ap(eset, si),
                                         start=(si == 0), stop=(si == nsi - 1))
                    rec = rcp_pool.tile([128, 512], f32, tag="rec", name="rec")
                    nc.vector.reciprocal(rec[:], den[:])
                    up = ups_pool.tile([128, 512], f32, tag="up",
                                       name=f"up{tb}{h}")
                    for si in range(nsi):
                        g, sv = si // 4, si % 4
                        nc.tensor.matmul(up[:], vG[g][:, sv, bass.ts(h, 128)],
                                         e_ap(eset, si), start=(si == 0),
                                         stop=(si == nsi - 1))
                    nc.vector.tensor_mul(uT[h][:, bass.ts(tb, 512)], up[:],
                                         rec[:])

                def attn_block(tb, qnext=None, ropes_mid=None):
                    """Full attention t-block. The PE stream runs 3 QK chains
                    ahead of the first den/AV pass so the scalar engine's exp
                    stream (573ns/slice vs the PE's 216) never gates the PE;
                    the next t-block's first Q chain covers the den(3) tail.
                    ropes_mid (the next block's second rope pair) is placed
                    after qk1 so the DVE processes this block's diagonal mask
                    adds first — exp of the diagonal pairs gates den()."""
                    attn_qk(tb, 0)
                    attn_qk(tb, 1)
                    if ropes_mid is not None:
                        ropes_mid()
                    attn_qk(tb, 2)
                    attn_dv(tb, 0)
                    attn_qk(tb, 3)
                    attn_dv(tb, 1)
                    attn_dv(tb, 2)
                    if qnext is not None:
                        qnext()
                    attn_dv(tb, 3)

                qps = q_mm(0, 0)
                q_rope(0, 0, qps)
                qps = q_mm(0, 1)
                q_rope(0, 1, qps)
                ropes_mid = None
                for tb in range(1, NB):
                    if tb + 1 < NB:
                        load_xq(tb + 1)
                    holder = {}

                    def qnext(tb=tb, holder=holder):
                        holder["qps"] = q_mm(tb, 0)

                    attn_block(tb - 1, qnext=qnext, ropes_mid=ropes_mid)
                    q_rope(tb, 0, holder["qps"])
                    qps1 = q_mm(tb, 1)

                    def ropes_mid(tb=tb, qps1=qps1):
                        q_rope(tb, 1, qps1)

                attn_block(NB - 1, ropes_mid=ropes_mid)

            # ---------------- Phase 4: output projection ----------------
            with nc.named_scope("out_proj"), \
                 tc.tile_pool(name="ops", bufs=2, space="PSUM") as ops_pool, \
                 tc.tile_pool(name="ob", bufs=3) as ob_pool:
                for tt in range(NT):
                    op = ops_pool.tile([128, E], f32, tag="op", name=f"op{tt}")
                    for ec in range(4):
                        for h in range(HL):
                            nc.tensor.matmul(
                                op[:, bass.ts(ec, 512)],
                                uT[h][:, bass.ts(tt, 128)],
                                wo[:, h, bass.ts(ec, 512)],
                                start=(h == 0), stop=(h == HL - 1))
                    ob = ob_pool.tile([128, E], bf16, tag="ob", name=f"ob{tt}")
                    nc.scalar.copy(ob[:], op[:])
                    # Alternate store queues: the 8MB output stream exceeds
                    # one queue's bandwidth over this phase's 50us window.
                    q = nc.sync if tt % 2 == 0 else nc.gpsimd
                    q.dma_start(out_d[tt], ob[:])

    nc.compile()
    return nc


def _get_module():
    if "nc" not in _NC_CACHE:
        _NC_CACHE["nc"] = _build_module()
    return _NC_CACHE["nc"]


def _host_prep(inputs_q, inputs_kv, positions, Wq, Wk, Wv, Wo):
    """Build the 8 per-core input maps (device-packed layouts, bf16)."""
    import ml_dtypes
    bf16 = ml_dtypes.bfloat16
    perm = np.concatenate([np.arange(0, D, 2), np.arange(1, D, 2)])  # de-interleave
    scale = np.float32(1.0 / np.sqrt(D))
    half = D // 2
    timescale = 10000.0 ** (2.0 * np.arange(half, dtype=np.float64) / D)
    ones = np.ones((128, 128), dtype=bf16)
    # mask variant v (diag sub-block at cols [128v, 128v+128)), width-trimmed:
    # masked (additive MASK_VALUE) where col < 128*v + row
    s_i = np.arange(128)[:, None]
    msk = np.concatenate(
        [np.where(np.arange(MW[v])[None, :] < 128 * v + s_i, MASK_VALUE, 0.0)
         for v in range(4)], axis=1).astype(np.float32)

    def pack_x(xT):
        # [E, S] f32 -> [NB, 128, NE, 512]: x_p[tb, p, et, t] = xT[128 et + p, 512 tb + t]
        return np.ascontiguousarray(
            xT.reshape(NE, 128, NB, 512).transpose(2, 1, 0, 3).astype(bf16))

    def pack_w(w):
        # [E, ND] -> [128, NE, ND]: w_p[p, et, n] = w[128 et + p, n]
        return np.ascontiguousarray(
            w.reshape(NE, 128, ND).transpose(1, 0, 2).astype(bf16))

    in_maps = []
    for c in range(8):
        b = c // 4
        h0 = (c % 4) * HL
        angle = positions[b].astype(np.float64)[None, :] / timescale[:, None]  # [64,S]
        cs = np.cos(angle).astype(np.float32)
        sn = np.sin(angle).astype(np.float32)
        csd = np.concatenate([cs, cs], axis=0)               # [128, S]
        sns = np.concatenate([-sn, sn], axis=0)              # [128, S]
        wq = (Wq[:, h0:h0 + HL, :][:, :, perm] * scale).reshape(E, ND)
        wk = Wk[:, h0:h0 + HL, :][:, :, perm].reshape(E, ND)
        wv = Wv[:, h0:h0 + HL, :].reshape(E, ND)
        wo = Wo[h0:h0 + HL]                                   # [HL, D, E]
        in_maps.append({
            "xq_p": pack_x(np.asarray(inputs_q[b]).T),
            "xkv_p": pack_x(np.asarray(inputs_kv[b]).T),
            "wq_p": pack_w(np.asarray(wq, dtype=np.float32)),
            "wk_p": pack_w(np.asarray(wk, dtype=np.float32)),
            "wv_p": pack_w(np.asarray(wv, dtype=np.float32)),
            "wo_p": np.ascontiguousarray(
                np.asarray(wo, dtype=np.float32).transpose(1, 0, 2).astype(bf16)),
            "csd": csd, "sns": sns, "ones": ones, "msk": msk,
        })
    return in_maps


def kernel(inputs_q, inputs_kv, positions, Wq, Wk, Wv, Wo, _trace=False,
           _trace_kwargs=None):
    from concourse import bass_utils

    nc = _get_module()
    in_maps = _host_prep(inputs_q, inputs_kv, positions, Wq, Wk, Wv, Wo)
    res = bass_utils.run_bass_kernel_spmd(
        nc, in_maps, core_ids=list(range(8)), trace=_trace,
        **(_trace_kwargs or {}))
    if _trace:
        _NC_CACHE["last_results"] = res
    parts = [np.asarray(res.results[c]["out"], dtype=np.float32).reshape(S, E)
             for c in range(8)]
    out0 = parts[0] + parts[1] + parts[2] + parts[3]
    out1 = parts[4] + parts[5] + parts[6] + parts[7]
    return np.stack([out0, out1]).astype(np.float32)


# revision 37
# speedup vs baseline: 1.0172x; 1.0063x over previous
"""Multi-head dot-product attention (RoPE, causal) on 8 NeuronCores.

Sharding: data-parallel over batch (2) x tensor-parallel over heads (16 -> 4
per core). Each core projects q/k/v for its 4 heads, runs causal attention,
and computes a partial output projection; the host sums the 4 partials per
batch element.

v2 design notes (vs the f32r baseline):
- All matmul operands are bf16 (PSUM accumulation stays f32): same PE
  throughput as f32r but half the DMA bytes and SBUF footprint. Host packs
  every DRAM tensor in the exact SBUF layout so all loads are full-line
  contiguous copies.
- Phase order: KV projection (all 4 t-blocks) -> per t-block [Q projection +
  attention of the previous t-block interleaved] -> output projection. The
  PE instruction stream never waits on a phase boundary: attention t-block
  tb only needs KV blocks <= tb and the Q block produced just before it.
- Attention keeps the transposed-scores layout: ST[s, t] so A@V needs no
  transposes, additive causal masks on the 4 diagonal sub-blocks only
  (width-trimmed), softmax denominator via an all-ones stationary matmul,
  reciprocal via Ln/Exp on the scalar engine (same activation table set as
  the softmax Exp), normalization during PSUM eviction.
- Cross-head interleave: the PE stream runs QK(h+1) between QK(h) and
  den/AV(h) so the scalar engine's exp latency is hidden; two eG buffers
  (even/odd head) break the WAR chain between consecutive heads.
- PSUM: q-projection accumulators share the attention score pool slots
  (3 x [128,2,512] = 6 banks) + den (1) + AV (1) = 8 banks exactly.
"""

import numpy as np

B, S, E, N, D = 2, 2048, 2048, 16, 128
HL = 4           # local heads per core (8 cores = 2 batch x 4 head groups)
ND = HL * D      # 512
NT = S // 128    # 16 row tiles
NB = S // 512    # 4 row blocks
NE = E // 128    # 16 contraction tiles
MASK_VALUE = float(-0.7 * np.finfo(np.float32).max)
MW = [128, 256, 384, 512]            # mask widths per diagonal variant
MOFF = [0, 128, 384, 768]            # col offsets of variants in msk table

_NC_CACHE = {}


def _build_module():
    import concourse.bass as bass
    import concourse.mybir as mybir
    import concourse.tile as tile
    from concourse import bacc

    f32 = mybir.dt.float32
    bf16 = mybir.dt.bfloat16
    Exp = mybir.ActivationFunctionType.Exp

    nc = bacc.Bacc("TRN2", target_bir_lowering=False, debug=False, num_devices=8)

    # Host-packed layouts (flat contiguous DMAs):
    xq_d = nc.dram_tensor("xq_p", [NB, 128, NE, 512], bf16, kind="ExternalInput").ap()
    xkv_d = nc.dram_tensor("xkv_p", [NB, 128, NE, 512], bf16, kind="ExternalInput").ap()
    wq_d = nc.dram_tensor("wq_p", [128, NE, ND], bf16, kind="ExternalInput").ap()
    wk_d = nc.dram_tensor("wk_p", [128, NE, ND], bf16, kind="ExternalInput").ap()
    wv_d = nc.dram_tensor("wv_p", [128, NE, ND], bf16, kind="ExternalInput").ap()
    wo_d = nc.dram_tensor("wo_p", [128, HL, E], bf16, kind="ExternalInput").ap()
    csd_d = nc.dram_tensor("csd", [128, S], f32, kind="ExternalInput").ap()
    sns_d = nc.dram_tensor("sns", [128, S], f32, kind="ExternalInput").ap()
    ones_d = nc.dram_tensor("ones", [128, 128], bf16, kind="ExternalInput").ap()
    msk_d = nc.dram_tensor("msk", [128, 1280], f32, kind="ExternalInput").ap()
    out_d = nc.dram_tensor("out", [NT, 128, E], bf16, kind="ExternalOutput").ap()

    with tile.TileContext(nc) as tc:
        with tc.tile_pool(name="const", bufs=1) as cpool, \
             tc.tile_pool(name="wqo", bufs=1) as wqo_pool, \
             tc.tile_pool(name="xq", bufs=2) as xq_pool, \
             tc.tile_pool(name="persist", bufs=1) as pers_pool:
            csd = cpool.tile([128, S], f32, tag="csd")
            sns = cpool.tile([128, S], f32, tag="sns")
            msk = cpool.tile([128, 1280], f32, tag="msk")
            ones = cpool.tile([128, 128], bf16, tag="ones")
            wq = wqo_pool.tile([128, NE, ND], bf16, tag="wq")
            wo = wqo_pool.tile([128, HL, E], bf16, tag="wo")
            kT = [pers_pool.tile([128, S], bf16, tag=f"kT{h}", name=f"kT{h}")
                  for h in range(HL)]
            vG = [pers_pool.tile([128, 4, ND], bf16, tag=f"vG{g}", name=f"vG{g}")
                  for g in range(NB)]
            uT = [pers_pool.tile([128, S], bf16, tag=f"uT{h}", name=f"uT{h}")
                  for h in range(HL)]
            xq_tiles = {}

            def load_xq(tb):
                xqt = xq_pool.tile([128, NE, 512], bf16, tag="xq",
                                   name=f"xq{tb}")
                nc.sync.dma_start(xqt[:].rearrange("p a b -> p (a b)"),
                                  xq_d[tb].rearrange("p a b -> p (a b)"))
                xq_tiles[tb] = xqt

            def rope(dst, src, tb, pool):
                """dst[:, tb-block] = rope(src) with de-interleaved head dim.
                src is a [128, 512] f32 PSUM AP; dst is bf16 SBUF."""
                tbs = bass.ts(tb, 512)
                tmp = pool.tile([128, 512], f32, tag="tmp", name="tmp")
                tmp2 = pool.tile([128, 512], f32, tag="tmp2", name="tmp2")
                nc.vector.tensor_mul(tmp[0:64, :], src[64:128, :], sns[0:64, tbs])
                nc.vector.tensor_mul(tmp[64:128, :], src[0:64, :], sns[64:128, tbs])
                nc.vector.tensor_mul(tmp2[:], src[:], csd[:, tbs])
                nc.vector.tensor_add(dst[:, tbs], tmp2[:], tmp[:])

            # ---------------- Phase 1: K + V projection ----------------
            with nc.named_scope("proj_kv"), \
                 tc.tile_pool(name="wkv", bufs=1) as wkv_pool, \
                 tc.tile_pool(name="xkv", bufs=2) as xkv_pool, \
                 tc.tile_pool(name="kvps", bufs=2, space="PSUM") as kvps_pool, \
                 tc.tile_pool(name="rope_kv", bufs=2) as rkv_pool:
                wk = wkv_pool.tile([128, NE, ND], bf16, tag="wk")
                wv = wkv_pool.tile([128, NE, ND], bf16, tag="wv")
                # Preloads. wk/wv interleaved chunks on the gpsimd queue (the
                # et loop consumes both in lockstep); tables on the scalar
                # queue in parallel; wq/wo behind wk/wv (needed later).
                # Flatten [p, a, b] -> [p, (a b)] on both sides: the DRAM and
                # SBUF runs are contiguous per partition, and 2D APs let the
                # descriptor generator emit 8-16KB descriptors instead of 1KB
                # (1KB descriptors cap HBM DMA at ~40% of peak).
                def fl(ap):
                    return ap.rearrange("p a b -> p (a b)")

                # Start-of-kernel DMA is the critical path: only wk/wv (gpsimd
                # queue) and xkv[0] (sync queue) compete for the engines; the
                # rope tables queue behind the weights, and everything not
                # needed before attention (wq, wo, msk, ones) is dispatched
                # one t-block later.
                nc.sync.dma_start(ones[:], ones_d[:])
                for ch in range(4):
                    nc.gpsimd.dma_start(fl(wk[:, 4 * ch:4 * (ch + 1), :]),
                                        fl(wk_d[:, 4 * ch:4 * (ch + 1), :]))
                    nc.gpsimd.dma_start(fl(wv[:, 4 * ch:4 * (ch + 1), :]),
                                        fl(wv_d[:, 4 * ch:4 * (ch + 1), :]))

                for tb in range(NB):
                    xk = xkv_pool.tile([128, NE, 512], bf16, tag="xk",
                                       name=f"xk{tb}")
                    if tb == 0:
                        for ch in range(4):
                            nc.sync.dma_start(
                                fl(xk[:, 4 * ch:4 * (ch + 1), :]),
                                fl(xkv_d[tb][:, 4 * ch:4 * (ch + 1), :]))
                        # rope tables ride the sync queue behind the first x
                        # block (the gpsimd queue is saturated with weights)
                        nc.sync.dma_start(csd[:], csd_d[:])
                        nc.sync.dma_start(sns[:], sns_d[:])
                    else:
                        nc.sync.dma_start(fl(xk[:]), fl(xkv_d[tb]))
                    if tb == 1:
                        nc.gpsimd.dma_start(fl(wq[:]), fl(wq_d[:]))
                        nc.gpsimd.dma_start(fl(wo[:]), fl(wo_d[:]))
                        nc.scalar.dma_start(msk[:], msk_d[:])
                    if tb == NB - 1:
                        # Dispatch the first two Q blocks behind the last xkv
                        # block on the sync queue so Q projection never waits.
                        load_xq(0)
                        load_xq(1)
                    for pp in range(2):   # 2 half-passes: 2 heads + 2 s-subtiles
                        kps = kvps_pool.tile([128, 2, 512], f32, tag="kps",
                                             name=f"kps{tb}{pp}")
                        vps = kvps_pool.tile([128, 2, 512], f32, tag="vps",
                                             name=f"vps{tb}{pp}")
                        if tb == 0 and pp == 0:
                            # Warm-up: the PE inevitably waits ~10us here for
                            # the first weight/x chunks. Chew dummy matmuls on
                            # the ones tile meanwhile so the HAM clock-gate is
                            # at 2.4GHz when the real chains start (the real
                            # et=0 matmul's start=True resets the bank).
                            for _ in range(40):
                                nc.tensor.matmul(kps[:, 0, 0:128], ones[:],
                                                 ones[:], start=True,
                                                 stop=True)
                        for et in range(NE):
                            for i in range(2):
                                h = 2 * pp + i
                                nc.tensor.matmul(
                                    kps[:, i], wk[:, et, bass.ts(h, 128)],
                                    xk[:, et, :], start=(et == 0),
                                    stop=(et == NE - 1))
                            for i in range(2):
                                sv = 2 * pp + i
                                nc.tensor.matmul(
                                    vps[:, i], xk[:, et, bass.ts(sv, 128)],
                                    wv[:, et, :], start=(et == 0),
                                    stop=(et == NE - 1))
                        for i in range(2):
                            rope(kT[2 * pp + i], kps[:, i], tb, rkv_pool)
                            nc.scalar.copy(vG[tb][:, 2 * pp + i, :], vps[:, i])

            # ---------- Phase 2+3: Q projection + attention, interleaved ----------
            with nc.named_scope("q_attn"), \
                 tc.tile_pool(name="qat", bufs=1) as qat_pool, \
                 tc.tile_pool(name="sps", bufs=3, space="PSUM") as sps_pool, \
                 tc.tile_pool(name="dps", bufs=1, space="PSUM") as dps_pool, \
                 tc.tile_pool(name="ups", bufs=1, space="PSUM") as ups_pool, \
                 tc.tile_pool(name="rope_q", bufs=1) as rq_pool, \
                 tc.tile_pool(name="rcp", bufs=2) as rcp_pool:
                qT = [qat_pool.tile([128, S], bf16, tag=f"qT{h}", name=f"qT{h}")
                      for h in range(HL)]
                # three eG sets, rotating h%3: lets QK of 3 heads run ahead
                # of the first den/AV pass without WAR serialization
                eG = [[qat_pool.tile([128, 2048], bf16, tag=f"eG{p}{g}",
                                     name=f"eG{p}{g}") for g in range(4)]
                      for p in range(3)]

                def e_ap(eset, si):
                    return eset[si // 4][:, bass.ds(512 * (si % 4), 512)]

                def q_mm(tb, hp):
                    """Project heads (2hp, 2hp+1) for t-block tb. Returns psum."""
                    qps = sps_pool.tile([128, 2, 512], f32, tag="sp",
                                        name=f"qps{tb}{hp}")
                    xqt = xq_tiles[tb]
                    for et in range(NE):
                        for i in range(2):
                            h = 2 * hp + i
                            nc.tensor.matmul(
                                qps[:, i], wq[:, et, bass.ts(h, 128)],
                                xqt[:, et, :], start=(et == 0),
                                stop=(et == NE - 1))
                    return qps

                def q_rope(tb, hp, qps):
                    for i in range(2):
                        rope(qT[2 * hp + i], qps[:, i], tb, rq_pool)

                def attn_qk(tb, h):
                    """Scores + exp for head h of t-block tb."""
                    nsi = 4 * (tb + 1)
                    eset = eG[h % 3]
                    for j in range(nsi // 2):
                        sp = sps_pool.tile([128, 2, 512], f32, tag="sp",
                                           name=f"sp{tb}{h}{j}")
                        for p2 in range(2):
                            si = 2 * j + p2
                            nc.tensor.matmul(
                                sp[:, p2], kT[h][:, bass.ts(si, 128)],
                                qT[h][:, bass.ts(tb, 512)], start=True,
                                stop=True)
                            v = si - 4 * tb
                            if v >= 0:
                                w = MW[v]
                                nc.vector.tensor_add(
                                    sp[:, p2, 0:w], sp[:, p2, 0:w],
                                    msk[:, bass.ds(MOFF[v], w)])
                        nc.scalar.activation(
                            eset[j // 2][:, bass.ts(j % 2, 1024)],
                            sp[:].rearrange("p a b -> p (a b)"), Exp)

                def attn_dv(tb, h):
                    """Denominator + A@V + normalization for head h."""
                    nsi = 4 * (tb + 1)
                    eset = eG[h % 3]
                    den = dps_pool.tile([128, 512], f32, tag="den",
                                        name=f"den{tb}{h}")
                    for si in range(nsi):
                        nc.tensor.matmul(den[:], ones[:], e_ap(eset, si),
                                         start=(si == 0), stop=(si == nsi - 1))
                    rec = rcp_pool.tile([128, 512], f32, tag="rec", name="rec")
                    nc.vector.reciprocal(rec[:], den[:])
                    up = ups_pool.tile([128, 512], f32, tag="up",
                                       name=f"up{tb}{h}")
                    for si in range(nsi):
                        g, sv = si // 4, si % 4
                        nc.tensor.matmul(up[:], vG[g][:, sv, bass.ts(h, 128)],
                                         e_ap(eset, si), start=(si == 0),
                                         stop=(si == nsi - 1))
                    nc.vector.tensor_mul(uT[h][:, bass.ts(tb, 512)], up[:],
                                         rec[:])

                def attn_block(tb, qnext=None, ropes_mid=None):
                    """Full attention t-block. The PE stream runs 3 QK chains
                    ahead of the first den/AV pass so the scalar engine's exp
                    stream (573ns/slice vs the PE's 216) never gates the PE;
                    the next t-block's first Q chain covers the den(3) tail.
                    ropes_mid (the next block's second rope pair) is placed
                    after qk1 so the DVE processes this block's diagonal mask
                    adds first — exp of the diagonal pairs gates den()."""
                    attn_qk(tb, 0)
                    attn_qk(tb, 1)
                    if ropes_mid is not None:
                        ropes_mid()
                    attn_qk(tb, 2)
                    attn_dv(tb, 0)
                    attn_qk(tb, 3)
                    attn_dv(tb, 1)
                    attn_dv(tb, 2)
                    if qnext is not None:
                        qnext()
                    attn_dv(tb, 3)

                qps = q_mm(0, 0)
                q_rope(0, 0, qps)
                qps = q_mm(0, 1)
                q_rope(0, 1, qps)
                for tb in range(1, NB):
                    if tb + 1 < NB:
                        load_xq(tb + 1)
                    holder = {}

                    def qnext(tb=tb, holder=holder):
                        holder["qps"] = q_mm(tb, 0)

                    attn_block(tb - 1, qnext=qnext)
                    q_rope(tb, 0, holder["qps"])
                    qps1 = q_mm(tb, 1)
                    q_rope(tb, 1, qps1)
                attn_block(NB - 1)

            # ---------------- Phase 4: output projection ----------------
            with nc.named_scope("out_proj"), \
                 tc.tile_pool(name="ops", bufs=2, space="PSUM") as ops_pool, \
                 tc.tile_pool(name="ob", bufs=3) as ob_pool:
                for tt in range(NT):
                    op = ops_pool.tile([128, E], f32, tag="op", name=f"op{tt}")
                    for ec in range(4):
                        for h in range(HL):
                            nc.tensor.matmul(
                                op[:, bass.ts(ec, 512)],
                                uT[h][:, bass.ts(tt, 128)],
                                wo[:, h, bass.ts(ec, 512)],
                                start=(h == 0), stop=(h == HL - 1))
                    ob = ob_pool.tile([128, E], bf16, tag="ob", name=f"ob{tt}")
                    nc.scalar.copy(ob[:], op[:])
                    # Alternate store queues: the 8MB output stream exceeds
                    # one queue's bandwidth over this phase's 50us window.
                    q = nc.sync if tt % 2 == 0 else nc.gpsimd
                    q.dma_start(out_d[tt], ob[:])

    nc.compile()
    return nc


def _get_module():
    if "nc" not in _NC_CACHE:
        _NC_CACHE["nc"] = _build_module()
    return _NC_CACHE["nc"]


def _host_prep(inputs_q, inputs_kv, positions, Wq, Wk, Wv, Wo):
    """Build the 8 per-core input maps (device-packed layouts, bf16)."""
    import ml_dtypes
    bf16 = ml_dtypes.bfloat16
    perm = np.concatenate([np.arange(0, D, 2), np.arange(1, D, 2)])  # de-interleave
    scale = np.float32(1.0 / np.sqrt(D))
    half = D // 2
    timescale = 10000.0 ** (2.0 * np.arange(half, dtype=np.float64) / D)
    ones = np.ones((128, 128), dtype=bf16)
    # mask variant v (diag sub-block at cols [128v, 128v+128)), width-trimmed:
    # masked (additive MASK_VALUE) where col < 128*v + row
    s_i = np.arange(128)[:, None]
    msk = np.concatenate(
        [np.where(np.arange(MW[v])[None, :] < 128 * v + s_i, MASK_VALUE, 0.0)
         for v in range(4)], axis=1).astype(np.float32)

    def pack_x(xT):
        # [E, S] f32 -> [NB, 128, NE, 512]: x_p[tb, p, et, t] = xT[128 et + p, 512 tb + t]
        return np.ascontiguousarray(
            xT.reshape(NE, 128, NB, 512).transpose(2, 1, 0, 3).astype(bf16))

    def pack_w(w):
        # [E, ND] -> [128, NE, ND]: w_p[p, et, n] = w[128 et + p, n]
        return np.ascontiguousarray(
            w.reshape(NE, 128, ND).transpose(1, 0, 2).astype(bf16))

    in_maps = []
    for c in range(8):
        b = c // 4
        h0 = (c % 4) * HL
        angle = positions[b].astype(np.float64)[None, :] / timescale[:, None]  # [64,S]
        cs = np.cos(angle).astype(np.float32)
        sn = np.sin(angle).astype(np.float32)
        csd = np.concatenate([cs, cs], axis=0)               # [128, S]
        sns = np.concatenate([-sn, sn], axis=0)              # [128, S]
        wq = (Wq[:, h0:h0 + HL, :][:, :, perm] * scale).reshape(E, ND)
        wk = Wk[:, h0:h0 + HL, :][:, :, perm].reshape(E, ND)
        wv = Wv[:, h0:h0 + HL, :].reshape(E, ND)
        wo = Wo[h0:h0 + HL]                                   # [HL, D, E]
        in_maps.append({
            "xq_p": pack_x(np.asarray(inputs_q[b]).T),
            "xkv_p": pack_x(np.asarray(inputs_kv[b]).T),
            "wq_p": pack_w(np.asarray(wq, dtype=np.float32)),
            "wk_p": pack_w(np.asarray(wk, dtype=np.float32)),
            "wv_p": pack_w(np.asarray(wv, dtype=np.float32)),
            "wo_p": np.ascontiguousarray(
                np.asarray(wo, dtype=np.float32).transpose(1, 0, 2).astype(bf16)),
            "csd": csd, "sns": sns, "ones": ones, "msk": msk,
        })
    return in_maps


def kernel(inputs_q, inputs_kv, positions, Wq, Wk, Wv, Wo, _trace=False,
           _trace_kwargs=None):
    from concourse import bass_utils

    nc = _get_module()
    in_maps = _host_prep(inputs_q, inputs_kv, positions, Wq, Wk, Wv, Wo)
    res = bass_utils.run_bass_kernel_spmd(
        nc, in_maps, core_ids=list(range(8)), trace=_trace,
        **(_trace_kwargs or {}))
    if _trace:
        _NC_CACHE["last_results"] = res
    parts = [np.asarray(res.results[c]["out"], dtype=np.float32).reshape(S, E)
             for c in range(8)]
    out0 = parts[0] + parts[1] + parts[2] + parts[3]
    out1 = parts[4] + parts[5] + parts[6] + parts[7]
    return np.stack([out0, out1]).astype(np.float32)


# revision 43
# speedup vs baseline: 1.0470x; 1.0293x over previous
"""Multi-head dot-product attention (RoPE, causal) on 8 NeuronCores.

Sharding: data-parallel over batch (2) x tensor-parallel over heads (16 -> 4
per core). Each core projects q/k/v for its 4 heads, runs causal attention,
and computes a partial output projection; the host sums the 4 partials per
batch element.

v2 design notes (vs the f32r baseline):
- All matmul operands are bf16 (PSUM accumulation stays f32): same PE
  throughput as f32r but half the DMA bytes and SBUF footprint. Host packs
  every DRAM tensor in the exact SBUF layout so all loads are full-line
  contiguous copies.
- Phase order: KV projection (all 4 t-blocks) -> per t-block [Q projection +
  attention of the previous t-block interleaved] -> output projection. The
  PE instruction stream never waits on a phase boundary: attention t-block
  tb only needs KV blocks <= tb and the Q block produced just before it.
- Attention keeps the transposed-scores layout: ST[s, t] so A@V needs no
  transposes, additive causal masks on the 4 diagonal sub-blocks only
  (width-trimmed), softmax denominator via an all-ones stationary matmul,
  reciprocal on the vector engine (Ln/Exp on ACT ping-pongs activation
  table sets at 2.7us per switch), normalization during PSUM eviction.
- Cross-head interleave: the PE stream runs QK(h+1) between QK(h) and
  den/AV(h) so the scalar engine's exp latency is hidden; two eG buffers
  (even/odd head) break the WAR chain between consecutive heads.
- PSUM: q-projection accumulators share the attention score pool slots
  (3 x [128,2,512] = 6 banks) + den (1) + AV (1) = 8 banks exactly.
"""

import numpy as np

B, S, E, N, D = 2, 2048, 2048, 16, 128
HL = 4           # local heads per core (8 cores = 2 batch x 4 head groups)
ND = HL * D      # 512
NT = S // 128    # 16 row tiles
NB = S // 512    # 4 row blocks
NE = E // 128    # 16 contraction tiles
MASK_VALUE = float(-0.7 * np.finfo(np.float32).max)
MW = [128, 256, 384, 512]            # mask widths per diagonal variant
MOFF = [0, 128, 384, 768]            # col offsets of variants in msk table

_NC_CACHE = {}


def _build_module():
    import concourse.bass as bass
    import concourse.mybir as mybir
    import concourse.tile as tile
    from concourse import bacc

    f32 = mybir.dt.float32
    bf16 = mybir.dt.bfloat16
    Exp = mybir.ActivationFunctionType.Exp

    nc = bacc.Bacc("TRN2", target_bir_lowering=False, debug=False, num_devices=8)

    # Host-packed layouts (flat contiguous DMAs):
    xq_d = nc.dram_tensor("xq_p", [NB, 128, NE, 512], bf16, kind="ExternalInput").ap()
    xkv_d = nc.dram_tensor("xkv_p", [NB, 128, NE, 512], bf16, kind="ExternalInput").ap()
    wq_d = nc.dram_tensor("wq_p", [128, NE, ND], bf16, kind="ExternalInput").ap()
    wk_d = nc.dram_tensor("wk_p", [128, NE, ND], bf16, kind="ExternalInput").ap()
    wv_d = nc.dram_tensor("wv_p", [128, NE, ND], bf16, kind="ExternalInput").ap()
    wo_d = nc.dram_tensor("wo_p", [128, HL, E], bf16, kind="ExternalInput").ap()
    csd_d = nc.dram_tensor("csd", [128, S], f32, kind="ExternalInput").ap()
    sns_d = nc.dram_tensor("sns", [128, S], f32, kind="ExternalInput").ap()
    ones_d = nc.dram_tensor("ones", [128, 128], bf16, kind="ExternalInput").ap()
    msk_d = nc.dram_tensor("msk", [128, 1280], f32, kind="ExternalInput").ap()
    out_d = nc.dram_tensor("out", [NT, 128, E], bf16, kind="ExternalOutput").ap()

    with tile.TileContext(nc) as tc:
        with tc.tile_pool(name="const", bufs=1) as cpool, \
             tc.tile_pool(name="wqo", bufs=1) as wqo_pool, \
             tc.tile_pool(name="xq", bufs=2) as xq_pool, \
             tc.tile_pool(name="persist", bufs=1) as pers_pool:
            csd = cpool.tile([128, S], f32, tag="csd")
            sns = cpool.tile([128, S], f32, tag="sns")
            msk = cpool.tile([128, 1280], f32, tag="msk")
            ones = cpool.tile([128, 128], bf16, tag="ones")
            wq = wqo_pool.tile([128, NE, ND], bf16, tag="wq")
            wo = wqo_pool.tile([128, HL, E], bf16, tag="wo")
            kT = [pers_pool.tile([128, S], bf16, tag=f"kT{h}", name=f"kT{h}")
                  for h in range(HL)]
            vG = [pers_pool.tile([128, 4, ND], bf16, tag=f"vG{g}", name=f"vG{g}")
                  for g in range(NB)]
            uT = [pers_pool.tile([128, S], bf16, tag=f"uT{h}", name=f"uT{h}")
                  for h in range(HL)]
            xq_tiles = {}

            def load_xq(tb):
                xqt = xq_pool.tile([128, NE, 512], bf16, tag="xq",
                                   name=f"xq{tb}")
                nc.sync.dma_start(xqt[:].rearrange("p a b -> p (a b)"),
                                  xq_d[tb].rearrange("p a b -> p (a b)"))
                xq_tiles[tb] = xqt

            def rope(dst, src, tb, pool):
                """dst[:, tb-block] = rope(src) with de-interleaved head dim.
                src is a [128, 512] f32 PSUM AP; dst is bf16 SBUF."""
                tbs = bass.ts(tb, 512)
                tmp = pool.tile([128, 512], f32, tag="tmp", name="tmp")
                tmp2 = pool.tile([128, 512], f32, tag="tmp2", name="tmp2")
                nc.vector.tensor_mul(tmp[0:64, :], src[64:128, :], sns[0:64, tbs])
                nc.vector.tensor_mul(tmp[64:128, :], src[0:64, :], sns[64:128, tbs])
                nc.vector.tensor_mul(tmp2[:], src[:], csd[:, tbs])
                nc.vector.tensor_add(dst[:, tbs], tmp2[:], tmp[:])

            # ---------------- Phase 1: K + V projection ----------------
            with nc.named_scope("proj_kv"), \
                 tc.tile_pool(name="wkv", bufs=1) as wkv_pool, \
                 tc.tile_pool(name="xkv", bufs=2) as xkv_pool, \
                 tc.tile_pool(name="kvps", bufs=2, space="PSUM") as kvps_pool, \
                 tc.tile_pool(name="rope_kv", bufs=2) as rkv_pool:
                wk = wkv_pool.tile([128, NE, ND], bf16, tag="wk")
                wv = wkv_pool.tile([128, NE, ND], bf16, tag="wv")
                # Preloads. wk/wv interleaved chunks on the gpsimd queue (the
                # et loop consumes both in lockstep); tables on the scalar
                # queue in parallel; wq/wo behind wk/wv (needed later).
                # Flatten [p, a, b] -> [p, (a b)] on both sides: the DRAM and
                # SBUF runs are contiguous per partition, and 2D APs let the
                # descriptor generator emit 8-16KB descriptors instead of 1KB
                # (1KB descriptors cap HBM DMA at ~40% of peak).
                def fl(ap):
                    return ap.rearrange("p a b -> p (a b)")

                # Start-of-kernel DMA is the critical path: only wk/wv (gpsimd
                # queue) and xkv[0] (sync queue) compete for the engines; the
                # rope tables queue behind the weights, and everything not
                # needed before attention (wq, wo, msk, ones) is dispatched
                # one t-block later.
                nc.sync.dma_start(ones[:], ones_d[:])
                for ch in range(4):
                    nc.gpsimd.dma_start(fl(wk[:, 4 * ch:4 * (ch + 1), :]),
                                        fl(wk_d[:, 4 * ch:4 * (ch + 1), :]))
                    nc.gpsimd.dma_start(fl(wv[:, 4 * ch:4 * (ch + 1), :]),
                                        fl(wv_d[:, 4 * ch:4 * (ch + 1), :]))

                for tb in range(NB):
                    xk = xkv_pool.tile([128, NE, 512], bf16, tag="xk",
                                       name=f"xk{tb}")
                    if tb == 0:
                        for ch in range(4):
                            nc.sync.dma_start(
                                fl(xk[:, 4 * ch:4 * (ch + 1), :]),
                                fl(xkv_d[tb][:, 4 * ch:4 * (ch + 1), :]))
                        # rope tables ride the sync queue behind the first x
                        # block (the gpsimd queue is saturated with weights);
                        # only the tb=0 columns are needed for the first ropes
                        nc.sync.dma_start(csd[:, 0:512], csd_d[:, 0:512])
                        nc.sync.dma_start(sns[:, 0:512], sns_d[:, 0:512])
                    else:
                        nc.sync.dma_start(fl(xk[:]), fl(xkv_d[tb]))
                    if tb == 1:
                        nc.sync.dma_start(csd[:, 512:], csd_d[:, 512:])
                        nc.sync.dma_start(sns[:, 512:], sns_d[:, 512:])
                        nc.gpsimd.dma_start(fl(wq[:]), fl(wq_d[:]))
                        nc.gpsimd.dma_start(fl(wo[:]), fl(wo_d[:]))
                        nc.scalar.dma_start(msk[:], msk_d[:])
                    if tb == NB - 1:
                        # Dispatch the first two Q blocks behind the last xkv
                        # block on the sync queue so Q projection never waits.
                        load_xq(0)
                        load_xq(1)
                    for pp in range(2):   # 2 half-passes: 2 heads + 2 s-subtiles
                        kps = kvps_pool.tile([128, 2, 512], f32, tag="kps",
                                             name=f"kps{tb}{pp}")
                        vps = kvps_pool.tile([128, 2, 512], f32, tag="vps",
                                             name=f"vps{tb}{pp}")
                        if tb == 0 and pp == 0:
                            # Warm-up: the PE inevitably waits ~10us here for
                            # the first weight/x chunks. Chew dummy matmuls on
                            # the ones tile meanwhile so the HAM clock-gate is
                            # at 2.4GHz when the real chains start (the real
                            # et=0 matmul's start=True resets the bank).
                            for _ in range(40):
                                nc.tensor.matmul(kps[:, 0, 0:128], ones[:],
                                                 ones[:], start=True,
                                                 stop=True)
                        for et in range(NE):
                            for i in range(2):
                                h = 2 * pp + i
                                nc.tensor.matmul(
                                    kps[:, i], wk[:, et, bass.ts(h, 128)],
                                    xk[:, et, :], start=(et == 0),
                                    stop=(et == NE - 1))
                            for i in range(2):
                                sv = 2 * pp + i
                                nc.tensor.matmul(
                                    vps[:, i], xk[:, et, bass.ts(sv, 128)],
                                    wv[:, et, :], start=(et == 0),
                                    stop=(et == NE - 1))
                        for i in range(2):
                            rope(kT[2 * pp + i], kps[:, i], tb, rkv_pool)
                            nc.scalar.copy(vG[tb][:, 2 * pp + i, :], vps[:, i])

            # ---------- Phase 2+3: Q projection + attention, interleaved ----------
            with nc.named_scope("q_attn"), \
                 tc.tile_pool(name="qat", bufs=1) as qat_pool, \
                 tc.tile_pool(name="sps", bufs=3, space="PSUM") as sps_pool, \
                 tc.tile_pool(name="dps", bufs=1, space="PSUM") as dps_pool, \
                 tc.tile_pool(name="ups", bufs=1, space="PSUM") as ups_pool, \
                 tc.tile_pool(name="rope_q", bufs=1) as rq_pool, \
                 tc.tile_pool(name="rcp", bufs=2) as rcp_pool:
                qT = [qat_pool.tile([128, S], bf16, tag=f"qT{h}", name=f"qT{h}")
                      for h in range(HL)]
                # three eG sets, rotating h%3: lets QK of 3 heads run ahead
                # of the first den/AV pass without WAR serialization
                eG = [[qat_pool.tile([128, 2048], bf16, tag=f"eG{p}{g}",
                                     name=f"eG{p}{g}") for g in range(4)]
                      for p in range(3)]

                def e_ap(eset, si):
                    return eset[si // 4][:, bass.ds(512 * (si % 4), 512)]

                def q_mm(tb, hp):
                    """Project heads (2hp, 2hp+1) for t-block tb. Returns psum."""
                    qps = sps_pool.tile([128, 2, 512], f32, tag="sp",
                                        name=f"qps{tb}{hp}")
                    xqt = xq_tiles[tb]
                    for et in range(NE):
                        for i in range(2):
                            h = 2 * hp + i
                            nc.tensor.matmul(
                                qps[:, i], wq[:, et, bass.ts(h, 128)],
                                xqt[:, et, :], start=(et == 0),
                                stop=(et == NE - 1))
                    return qps

                def q_rope(tb, hp, qps):
                    for i in range(2):
                        rope(qT[2 * hp + i], qps[:, i], tb, rq_pool)

                def attn_qk(tb, h):
                    """Scores + exp for head h of t-block tb. Diagonal pairs
                    first: their exp depends on DVE mask adds, so giving the
                    in-order DVE stream maximal slack hides rope/recip
                    backlog; den/AV accumulate them first symmetrically."""
                    nsi = 4 * (tb + 1)
                    eset = eG[h % 3]
                    porder = list(range(2 * tb, nsi // 2)) + list(range(2 * tb))
                    for j in porder:
                        sp = sps_pool.tile([128, 2, 512], f32, tag="sp",
                                           name=f"sp{tb}{h}{j}")
                        for p2 in range(2):
                            si = 2 * j + p2
                            nc.tensor.matmul(
                                sp[:, p2], kT[h][:, bass.ts(si, 128)],
                                qT[h][:, bass.ts(tb, 512)], start=True,
                                stop=True)
                            v = si - 4 * tb
                            if v >= 0:
                                w = MW[v]
                                nc.vector.tensor_add(
                                    sp[:, p2, 0:w], sp[:, p2, 0:w],
                                    msk[:, bass.ds(MOFF[v], w)])
                        nc.scalar.activation(
                            eset[j // 2][:, bass.ts(j % 2, 1024)],
                            sp[:].rearrange("p a b -> p (a b)"), Exp)

                def attn_dv(tb, h):
                    """Denominator + A@V + normalization for head h."""
                    nsi = 4 * (tb + 1)
                    eset = eG[h % 3]
                    sorder = list(range(4 * tb, nsi)) + list(range(4 * tb))
                    den = dps_pool.tile([128, 512], f32, tag="den",
                                        name=f"den{tb}{h}")
                    for i, si in enumerate(sorder):
                        nc.tensor.matmul(den[:], ones[:], e_ap(eset, si),
                                         start=(i == 0), stop=(i == nsi - 1))
                    rec = rcp_pool.tile([128, 512], f32, tag="rec", name="rec")
                    nc.vector.reciprocal(rec[:], den[:])
                    up = ups_pool.tile([128, 512], f32, tag="up",
                                       name=f"up{tb}{h}")
                    for i, si in enumerate(sorder):
                        g, sv = si // 4, si % 4
                        nc.tensor.matmul(up[:], vG[g][:, sv, bass.ts(h, 128)],
                                         e_ap(eset, si), start=(i == 0),
                                         stop=(i == nsi - 1))
                    nc.vector.tensor_mul(uT[h][:, bass.ts(tb, 512)], up[:],
                                         rec[:])

                def attn_block(tb, qnext=None, ropes_mid=None):
                    """Full attention t-block. The PE stream runs 3 QK chains
                    ahead of the first den/AV pass so the scalar engine's exp
                    stream (573ns/slice vs the PE's 216) never gates the PE;
                    the next t-block's first Q chain covers the den(3) tail.
                    ropes_mid (the next block's second rope pair) is placed
                    after qk1 so the DVE processes this block's diagonal mask
                    adds first — exp of the diagonal pairs gates den()."""
                    attn_qk(tb, 0)
                    attn_qk(tb, 1)
                    if ropes_mid is not None:
                        ropes_mid()
                    attn_qk(tb, 2)
                    attn_dv(tb, 0)
                    attn_qk(tb, 3)
                    attn_dv(tb, 1)
                    attn_dv(tb, 2)
                    if qnext is not None:
                        qnext()
                    attn_dv(tb, 3)

                qps = q_mm(0, 0)
                q_rope(0, 0, qps)
                qps = q_mm(0, 1)
                q_rope(0, 1, qps)
                for tb in range(1, NB):
                    if tb + 1 < NB:
                        load_xq(tb + 1)
                    holder = {}

                    def qnext(tb=tb, holder=holder):
                        holder["qps"] = q_mm(tb, 0)

                    attn_block(tb - 1, qnext=qnext)
                    q_rope(tb, 0, holder["qps"])
                    qps1 = q_mm(tb, 1)
                    q_rope(tb, 1, qps1)
                attn_block(NB - 1)

            # ---------------- Phase 4: output projection ----------------
            with nc.named_scope("out_proj"), \
                 tc.tile_pool(name="ops", bufs=2, space="PSUM") as ops_pool, \
                 tc.tile_pool(name="ob", bufs=3) as ob_pool:
                for tt in range(NT):
                    op = ops_pool.tile([128, E], f32, tag="op", name=f"op{tt}")
                    for ec in range(4):
                        for h in range(HL):
                            nc.tensor.matmul(
                                op[:, bass.ts(ec, 512)],
                                uT[h][:, bass.ts(tt, 128)],
                                wo[:, h, bass.ts(ec, 512)],
                                start=(h == 0), stop=(h == HL - 1))
                    ob = ob_pool.tile([128, E], bf16, tag="ob", name=f"ob{tt}")
                    nc.scalar.copy(ob[:], op[:])
                    # Alternate store queues: the 8MB output stream exceeds
                    # one queue's bandwidth over this phase's 50us window.
                    q = nc.sync if tt % 2 == 0 else nc.gpsimd
                    q.dma_start(out_d[tt], ob[:])

    nc.compile()
    return nc


def _get_module():
    if "nc" not in _NC_CACHE:
        _NC_CACHE["nc"] = _build_module()
    return _NC_CACHE["nc"]


def _host_prep(inputs_q, inputs_kv, positions, Wq, Wk, Wv, Wo):
    """Build the 8 per-core input maps (device-packed layouts, bf16)."""
    import ml_dtypes
    bf16 = ml_dtypes.bfloat16
    perm = np.concatenate([np.arange(0, D, 2), np.arange(1, D, 2)])  # de-interleave
    scale = np.float32(1.0 / np.sqrt(D))
    half = D // 2
    timescale = 10000.0 ** (2.0 * np.arange(half, dtype=np.float64) / D)
    ones = np.ones((128, 128), dtype=bf16)
    # mask variant v (diag sub-block at cols [128v, 128v+128)), width-trimmed:
    # masked (additive MASK_VALUE) where col < 128*v + row
    s_i = np.arange(128)[:, None]
    msk = np.concatenate(
        [np.where(np.arange(MW[v])[None, :] < 128 * v + s_i, MASK_VALUE, 0.0)
         for v in range(4)], axis=1).astype(np.float32)

    def pack_x(xT):
        # [E, S] f32 -> [NB, 128, NE, 512]: x_p[tb, p, et, t] = xT[128 et + p, 512 tb + t]
        return np.ascontiguousarray(
            xT.reshape(NE, 128, NB, 512).transpose(2, 1, 0, 3).astype(bf16))

    def pack_w(w):
        # [E, ND] -> [128, NE, ND]: w_p[p, et, n] = w[128 et + p, n]
        return np.ascontiguousarray(
            w.reshape(NE, 128, ND).transpose(1, 0, 2).astype(bf16))

    in_maps = []
    for c in range(8):
        b = c // 4
        h0 = (c % 4) * HL
        angle = positions[b].astype(np.float64)[None, :] / timescale[:, None]  # [64,S]
        cs = np.cos(angle).astype(np.float32)
        sn = np.sin(angle).astype(np.float32)
        csd = np.concatenate([cs, cs], axis=0)               # [128, S]
        sns = np.concatenate([-sn, sn], axis=0)              # [128, S]
        wq = (Wq[:, h0:h0 + HL, :][:, :, perm] * scale).reshape(E, ND)
        wk = Wk[:, h0:h0 + HL, :][:, :, perm].reshape(E, ND)
        wv = Wv[:, h0:h0 + HL, :].reshape(E, ND)
        wo = Wo[h0:h0 + HL]                                   # [HL, D, E]
        in_maps.append({
            "xq_p": pack_x(np.asarray(inputs_q[b]).T),
            "xkv_p": pack_x(np.asarray(inputs_kv[b]).T),
            "wq_p": pack_w(np.asarray(wq, dtype=np.float32)),
            "wk_p": pack_w(np.asarray(wk, dtype=np.float32)),
            "wv_p": pack_w(np.asarray(wv, dtype=np.float32)),
            "wo_p": np.ascontiguousarray(
                np.asarray(wo, dtype=np.float32).transpose(1, 0, 2).astype(bf16)),
            "csd": csd, "sns": sns, "ones": ones, "msk": msk,
        })
    return in_maps


def kernel(inputs_q, inputs_kv, positions, Wq, Wk, Wv, Wo, _trace=False,
           _trace_kwargs=None):
    from concourse import bass_utils

    nc = _get_module()
    in_maps = _host_prep(inputs_q, inputs_kv, positions, Wq, Wk, Wv, Wo)
    res = bass_utils.run_bass_kernel_spmd(
        nc, in_maps, core_ids=list(range(8)), trace=_trace,
        **(_trace_kwargs or {}))
    if _trace:
        _NC_CACHE["last_results"] = res
    parts = [np.asarray(res.results[c]["out"], dtype=np.float32).reshape(S, E)
             for c in range(8)]
    out0 = parts[0] + parts[1] + parts[2] + parts[3]
    out1 = parts[4] + parts[5] + parts[6] + parts[7]
    return np.stack([out0, out1]).astype(np.float32)
